# revision 4
# baseline (speedup 1.0000x reference)
"""Mixtral router aux-loss kernel for 8 Trainium2 NeuronCores.

Strategy (data-parallel over tokens, per the sharding hint):
  - Shard the 4194304-token gate_logits across 8 cores (524288 each).
  - Per core, stream the [524288, 8] f32 shard in natural token-major layout
    ([128 partitions, W tokens, 8 experts] tiles):
      * ScalarE: y = exp(x) in bf16 (logits ~N(0,1): no max-subtract needed),
        and r = 1/s via exp(-ln s) (vector reciprocal is slow).
      * VectorE (all bf16 tensor_tensor, 2x mode): tournament trees give the
        per-token sum s and 2nd-max m2; top-2 indicator ind = (y >= m2) via a
        pair-packed m2 so every compare stays in 2x mode. Everything past the
        first tree level runs on fused PAIRS of DMA tiles to halve the
        per-instruction overhead (58-cycle init + drain) of the small ops.
      * TensorE: per-expert contractions as PSUM-accumulated matmuls —
        counts: ones[128,1]^T @ ind-chunk; probs: r-block[128,64]^T @ y-chunk
        (the (w,e)-diagonal of the [64,512] product is sum_t y*r per expert,
        folded on the host).
  - Host gathers tiny [65, 512] partials per core, extracts the diagonal,
    rescales counts so sum(cnt) = 2T exactly (bf16 ties overcount ~0.5% of
    tokens by one; ties are index-symmetric so a global rescale is exact to
    O(1e-6) on the loss), and forms the final scalar.
"""

import sys

if "/opt/trn_rl_repo" not in sys.path:
    sys.path.insert(0, "/opt/trn_rl_repo")

import numpy as np

T_TOTAL = 4194304
E = 8
N_CORES = 8
TC = T_TOTAL // N_CORES  # tokens per core
P = 128  # SBUF partitions
W = 512  # tokens per partition per DMA tile
V = 2 * W  # tokens per partition per fused processing pair
NTILES = TC // (P * W)
NPAIRS = NTILES // 2
CHUNK_W = 64  # tokens per prob-matmul chunk (N = CHUNK_W * E = 512)
NCHUNK = V // CHUNK_W
AUX_LOSS_COEF = 0.02

_CACHE: dict = {}
LAST_RESULTS = None  # BassKernelResults of the most recent run (for test.py)


def _build_program(stage: int = 99, reps: int = 1):
    """stage: 0=DMA+exp, 1=+L1 trees, 2=+full trees/r, 3=+compare,
    4=+matmuls (full kernel). Lower stages are for sim ablations."""
    import concourse.bass as bass  # noqa: F401
    import concourse.tile as tile
    from concourse import bacc, mybir

    f32 = mybir.dt.float32
    bf16 = mybir.dt.bfloat16
    Alu = mybir.AluOpType
    Act = mybir.ActivationFunctionType

    # Force every activation onto the combined ln+exp table so bacc emits a
    # single InstLoadActFuncSet instead of thrashing Exp<->Ln tables per pair.
    # Other set entries are emptied (not removed) so act_func_set_id indices
    # stay aligned with act_info.json.
    from concourse import bacc as _bacc_mod, hw_specs as _hw
    _orig_tables = _hw.get_activation_tables

    def _patched_tables(arch):
        keep = "natural_log_exp_and_others"
        d = _orig_tables(arch)
        if keep not in d:
            return d
        return {k: (v if k == keep else set()) for k, v in d.items()}

    _bacc_mod.get_activation_tables = _patched_tables

    nc = bacc.Bacc("TRN2", target_bir_lowering=False, debug=False,
                   num_devices=N_CORES)
    def eng(name):  # all elementwise ops stay on the (fast) vector engine
        return nc.vector
    x = nc.dram_tensor("x", [TC, E], f32, kind="ExternalInput")
    out = nc.dram_tensor("out", [CHUNK_W + 1, CHUNK_W * E], f32,
                         kind="ExternalOutput")
    outc = nc.dram_tensor("outc", [P, E], f32, kind="ExternalOutput")

    # [NPAIRS, 2, 128, W, 8]; half h of pair n lands in yt[:, h*W:(h+1)*W].
    xr2 = x.ap().rearrange("(n h p w) e -> n h p w e", h=2, p=P, w=W)
    xr4 = x.ap().rearrange("(n h p w) e -> n h p w e", h=4, p=P, w=V // 4)

    with tile.TileContext(nc) as tc:
        with (
            tc.tile_pool(name="dbuf", bufs=2) as dbuf,
            tc.tile_pool(name="tree", bufs=2) as tree,
            tc.tile_pool(name="sing", bufs=1) as sing,
            tc.tile_pool(name="psum", bufs=1, space="PSUM") as psump,
        ):
            ones = sing.tile([P, 1], bf16)
            nc.vector.memset(ones, 1.0)
            psum_cnt = psump.tile([1, CHUNK_W * E], f32)
            psum_prob = psump.tile([CHUNK_W, CHUNK_W * E], f32)
            cnt_acc = sing.tile([P, E], f32)
            nc.vector.memset(cnt_acc, 0.0)

            for rep in range(reps):
              for n in range(NPAIRS):
                first = rep == 0 and n == 0
                last = rep == reps - 1 and n == NPAIRS - 1

                # Two DMA tiles -> one fused bf16 pair tile (quarters
                # for the very first pair to shorten the pipeline fill)
                yt = dbuf.tile([P, V, E], bf16, tag="yt")
                nsplit = 4 if (rep == 0 and n == 0) else 2
                step = V // nsplit
                for h in range(nsplit):
                    xt = dbuf.tile([P, step, E], f32, tag="xt")
                    nc.sync.dma_start(
                        xt[:], xr4[n, h] if nsplit == 4 else xr2[n, h])
                    nc.scalar.activation(
                        yt[:, h * step:(h + 1) * step, :], xt[:], Act.Exp)
                if stage < 1:
                    continue

                # Level 1: pair groups {i, i+4}, split per half so the
                # DVE starts as soon as the first half's exp lands.
                P4 = tree.tile([P, V, 4], bf16, tag="P4")
                Q4 = tree.tile([P, V, 4], bf16, tag="Q4")
                S4 = sing.tile([P, V, 4], bf16, tag="S4")
                s96 = None
                if stage == 96:
                    s96 = tree.tile([P, V], f32, tag="s")
                for h in range(2):
                    sl = slice(h * W, (h + 1) * W)
                    Ah = yt[:, sl, 0:4]
                    Bh = yt[:, sl, 4:8]
                    nc.vector.tensor_tensor(P4[:, sl, :], Ah, Bh, op=Alu.max)
                    nc.vector.tensor_tensor(Q4[:, sl, :], Ah, Bh, op=Alu.min)
                    if stage == 96:
                        # one grouped reduce replaces the 3-op sum tree
                        nc.vector.tensor_reduce(s96[:, sl], yt[:, sl, :],
                                                axis=mybir.AxisListType.X,
                                                op=Alu.add)
                    else:
                        nc.vector.tensor_tensor(S4[:, sl, :], Ah, Bh,
                                                op=Alu.add)
                if stage < 2:
                    continue

                # Level 2: quads from pair-halves (contiguous operands, 2x)
                M2 = tree.tile([P, V, 2], bf16, tag="M2")
                nc.vector.tensor_tensor(M2[:], P4[:, :, 0:2], P4[:, :, 2:4],
                                        op=Alu.max)
                T2 = tree.tile([P, V, 2], bf16, tag="T2")
                eng('T2').tensor_tensor(T2[:], P4[:, :, 0:2], P4[:, :, 2:4],
                                        op=Alu.min)
                q2 = tree.tile([P, V, 2], bf16, tag="q2")
                eng('q2').tensor_tensor(q2[:], Q4[:, :, 0:2], Q4[:, :, 2:4],
                                        op=Alu.max)
                m2q = tree.tile([P, V, 2], bf16, tag="m2q")
                eng('m2q').tensor_tensor(m2q[:], T2[:], q2[:], op=Alu.max)
                S2 = sing.tile([P, V, 2], bf16, tag="S2")
                if stage != 96:
                    nc.vector.tensor_tensor(S2[:], S4[:, :, 0:2],
                                            S4[:, :, 2:4], op=Alu.add)

                # Level 3 (step-2 operands, 1x mode, small)
                if stage == 96:
                    s = s96
                else:
                    s = tree.tile([P, V], f32, tag="s")
                    nc.vector.tensor_tensor(s[:], S2[:, :, 0:1].squeeze(2),
                                            S2[:, :, 1:2].squeeze(2),
                                            op=Alu.add)
                T3 = sing.tile([P, V], bf16, tag="T3")
                eng('T3').tensor_tensor(T3[:], M2[:, :, 0:1].squeeze(2),
                                        M2[:, :, 1:2].squeeze(2), op=Alu.min)
                q3 = sing.tile([P, V], bf16, tag="q3")
                eng('q3').tensor_tensor(q3[:], m2q[:, :, 0:1].squeeze(2),
                                        m2q[:, :, 1:2].squeeze(2), op=Alu.max)
                # m2 written twice into adjacent slots for pair-broadcast
                m2p = tree.tile([P, V, 2], bf16, tag="m2p")
                nc.vector.tensor_tensor(m2p[:, :, 0:1].squeeze(2), T3[:],
                                        q3[:], op=Alu.max)
                eng('m2pcopy').tensor_copy(m2p[:, :, 1:2].squeeze(2),
                                           m2p[:, :, 0:1].squeeze(2))

                # r = 1/s = exp(-ln s) on ScalarE (bf16 out for PE weights)
                nc.scalar.activation(s[:], s[:], Act.Ln)
                r = dbuf.tile([P, V], bf16, tag="r")
                nc.scalar.activation(r[:], s[:], Act.Exp, scale=-1.0)

                if stage < 3:
                    continue
                # Top-2 indicator, 2 experts at a time against packed m2.
                # stage 97: emit per half so half-0's matmuls start sooner.
                ind = dbuf.tile([P, V, E], bf16, tag="ind")
                csplit = 2 if stage == 97 else 1
                for h2 in range(csplit):
                    cs = slice(h2 * (V // csplit), (h2 + 1) * (V // csplit))
                    for i in range(4):
                        nc.vector.tensor_tensor(
                            ind[:, cs, 2 * i:2 * i + 2],
                            yt[:, cs, 2 * i:2 * i + 2], m2p[:, cs, :],
                            op=Alu.is_ge)
                    if stage == 97:
                        for c in range(h2 * NCHUNK // 2, (h2 + 1) * NCHUNK // 2):
                            rhs_ind = ind[:, c * CHUNK_W:(c + 1) * CHUNK_W, :]
                            nc.tensor.matmul(
                                psum_cnt[:], ones[:], rhs_ind,
                                start=(first and c == 0),
                                stop=(last and c == NCHUNK - 1))
                            rhs_y = yt[:, c * CHUNK_W:(c + 1) * CHUNK_W, :]
                            lhs_r = r[:, c * CHUNK_W:(c + 1) * CHUNK_W]
                            nc.tensor.matmul(
                                psum_prob[:], lhs_r, rhs_y,
                                start=(first and c == 0),
                                stop=(last and c == NCHUNK - 1))
                if stage == 97:
                    continue

                if stage < 4:
                    continue
                if stage == 98:
                    # counts on DVE: strided grouped reduce over w + accumulate
                    cnt_t = tree.tile([P, E], f32, tag="cnt_t")
                    nc.vector.tensor_reduce(
                        cnt_t[:], ind[:].rearrange("p w e -> p e w"),
                        axis=mybir.AxisListType.X, op=Alu.add)
                    nc.vector.tensor_tensor(cnt_acc[:], cnt_acc[:], cnt_t[:],
                                            op=Alu.add)
                # Per-expert contractions on the PE, accumulated in PSUM
                for c in range(NCHUNK):
                    if stage != 98:
                        rhs_ind = ind[:, c * CHUNK_W:(c + 1) * CHUNK_W, :]
                        nc.tensor.matmul(
                            psum_cnt[:], ones[:], rhs_ind,
                            start=(first and c == 0),
                            stop=(last and c == NCHUNK - 1))
                    rhs_y = yt[:, c * CHUNK_W:(c + 1) * CHUNK_W, :]
                    lhs_r = r[:, c * CHUNK_W:(c + 1) * CHUNK_W]
                    nc.tensor.matmul(
                        psum_prob[:], lhs_r, rhs_y,
                        start=(first and c == 0),
                        stop=(last and c == NCHUNK - 1))

            cnt_sb = sing.tile([1, CHUNK_W * E], f32)
            prob_sb = sing.tile([CHUNK_W, CHUNK_W * E], f32)
            if stage >= 4:
                if stage == 98:
                    nc.vector.memset(cnt_sb, 0.0)
                else:
                    nc.vector.tensor_copy(cnt_sb[:], psum_cnt[:])
                nc.vector.tensor_copy(prob_sb[:], psum_prob[:])
            else:
                nc.vector.memset(cnt_sb, 0.0)
                nc.vector.memset(prob_sb, 0.0)
            nc.gpsimd.dma_start(out.ap()[CHUNK_W:CHUNK_W + 1, :], cnt_sb[:])
            nc.gpsimd.dma_start(out.ap()[0:CHUNK_W, :], prob_sb[:])
            nc.gpsimd.dma_start(outc.ap(), cnt_acc[:])

    nc.compile()
    return nc


def kernel(gate_logits):
    global LAST_RESULTS
    from concourse.bass_utils import run_bass_kernel_spmd

    gl = np.asarray(gate_logits, dtype=np.float32)
    assert gl.shape == (T_TOTAL, E), gl.shape

    if "nc" not in _CACHE:
        _CACHE["nc"] = _build_program()
    nc = _CACHE["nc"]

    shards = gl.reshape(N_CORES, TC, E)
    in_maps = [{"x": np.ascontiguousarray(shards[i])} for i in range(N_CORES)]
    res = run_bass_kernel_spmd(nc, in_maps, core_ids=list(range(N_CORES)))
    LAST_RESULTS = res

    cnt = np.zeros(E, dtype=np.float64)
    prob = np.zeros(E, dtype=np.float64)
    for rmap in res.results:
        o = rmap["out"].astype(np.float64)
        # counts: DVE accumulator if present, else PSUM row (w % CHUNK_W, e)
        oc = rmap.get("outc")
        if oc is not None and float(np.abs(oc).sum()) > 0:
            cnt += oc.astype(np.float64).sum(axis=0)
        else:
            cnt += o[CHUNK_W].reshape(CHUNK_W, E).sum(axis=0)
        # probs: diagonal w' == (w % CHUNK_W) of [w', (w, e)]
        pr = o[0:CHUNK_W].reshape(CHUNK_W, CHUNK_W, E)
        prob += np.einsum("wwe->e", pr)

    # bf16 ties at the top-2 boundary triple-count a few tokens; ties are
    # index-symmetric, so rescaling to the exact total removes the bias.
    cnt *= (2.0 * T_TOTAL) / cnt.sum()

    tokens_per_expert = cnt / T_TOTAL
    router_prob_per_expert = prob / T_TOTAL
    loss = AUX_LOSS_COEF * float(
        np.sum(tokens_per_expert * router_prob_per_expert)) * E
    return np.float32(loss)

